# revision 55
# baseline (speedup 1.0000x reference)
"""Trainium2 Bass kernel for the Guided-Conv problem.

Math (per independent sample n, of NB = 4096):
  g_n, d_n : 24x24x9 patches of guidance / depth.
  c_n      = conv2d(g_n, conv_w, stride 8, SAME) + conv_b        -> 3x3x9
  k_n[i]   = c_n[:, :, i] / max(||c_n[:, :, i]||_2, 1)           (per-channel 3x3 filter)
  gap_n    = mean(g_n, (y, x))                                   -> 9
  W2_n     = (gap_n @ dense_w + dense_b).reshape(9, 9)           (i2 -> o2)
  r2_n[o]  = 1 / max(||W2_n[:, o]||_2, 1)
  out_n    = (depthwise(d_n, k_n) @ W2_n) * r2_n                 -> 24x24x9

Device strategy (per core: 512 samples + 6 pad = 37 groups of 14):
  Partition layout q = n_local*9 + ch on 126 partitions; free = pixels.
  - Kernel generation (c_n, W2_n) via block-diagonal fp16 matmuls:
    lhsT = kron(eye(14), w) built on host, 14 samples per matmul.
    dense weights are scaled x64 on host so fp16 never goes subnormal;
    the x64 cancels exactly in r2 via max(s1, 64) instead of max(.., 1).
  - gap: two-stage fp16 row/column reduces on Vector (partial sums stay
    small, so fp16 rounding is negligible at the 2e-2 tolerance).
  - Depthwise(3x3) + 1x1 fused: out[(n,o), pix] = sum_{t,i} BD_t[(n,i),(n,o)]
    * d_pad[(n,i), pix+t], 9 tap-matmuls accumulating in PSUM (fp16 inputs).
    BD = w2m (.) k broadcast, built with two fp16 tensor_tensor ops per group
    (per-partition-scalar PTR mode is ~2us/op on HW -- avoided).
  - r2 applied as the per-partition ACT scale on a single 2-bank PSUM->SBUF
    copy per group (matmul halves write word 0 / 512 of one PSUM tile).
  - Engine in-order queues: weight-gen for supertile s+1 is EMITTED inside
    the conv pair loop of supertile s, so it executes concurrently instead
    of stalling the PE at each supertile boundary.
  - ALL input DMAs issue up front: packed consts on Sync, guidance
    supertiles on Scalar, depth pairs on Sync, outputs (fp16) on GpSimd.
Host does all layout (patch extraction, channel de-interleave, zero-pad,
fp16 casts) -- every DMA is contiguous >=1KB runs per partition.
"""

import numpy as np

import concourse.bass as bass
from concourse import bacc
import concourse.mybir as mybir
from concourse.tile import TileContext
from concourse.bass_utils import run_bass_kernel_spmd

F = 9          # channels
P = 24         # patch size
PADW = 26      # padded patch width (SAME conv, pad 1)
KS = 3         # generated kernel size
NCORES = 8
NL = 14        # samples per group
Q = NL * F     # 126 used partitions
NGROUP = 37    # groups per core (36 full + 1 padded)
SPC = NGROUP * NL  # 518 sample slots per core (512 real)
PIX = P * P        # 576
PPIX = PADW * PADW  # 676
HALF = PIX // 2    # 288, pixels per PSUM chunk (<=512 fp32/bank)
SUPER = [4, 11, 11, 11]  # weight-gen supertile sizes (sum = 37)
NGMAX = max(SUPER)
DSCALE = 64.0  # host scales dense weights by this to keep fp16 normal

# packed-consts layout, in fp32 words per partition
WA = (KS * KS * Q) // 2     # lhsA  fp16 -> 567 words
WD = (KS * KS * Q) // 2     # lhsD  fp16
WM = Q // 2                 # mask  fp16 -> 63 words
OA = 0
OD = OA + WA
OD2 = OD + WD
OM = OD2 + WD
OB = OM + WM
CW = OB + 1

F32 = mybir.dt.float32
F16 = mybir.dt.float16


def build_program():
    nc = bacc.Bacc("TRN2", target_bir_lowering=False, debug=False,
                   num_devices=NCORES)

    gin = nc.dram_tensor("gin", [Q, NGROUP, PIX], F16, kind="ExternalInput").ap()
    din = nc.dram_tensor("din", [Q, NGROUP, PPIX], F16, kind="ExternalInput").ap()
    cst = nc.dram_tensor("consts", [128, CW], F32, kind="ExternalInput").ap()
    outd = nc.dram_tensor("out", [Q, NGROUP, PIX], F16, kind="ExternalOutput").ap()

    ID = mybir.ActivationFunctionType.Identity
    CP = mybir.ActivationFunctionType.Copy
    AX = mybir.AxisListType.X
    ADD = mybir.AluOpType.add
    MUL = mybir.AluOpType.mult

    with TileContext(nc) as tc:
        with (
            tc.tile_pool(name="consts", bufs=1) as cpool,
            tc.tile_pool(name="gpool", bufs=4) as gpool,
            tc.tile_pool(name="dpool", bufs=8) as dpool,
            tc.tile_pool(name="opool", bufs=8) as opool,
            tc.tile_pool(name="small", bufs=1) as spool,
            tc.tile_pool(name="gap", bufs=2) as gappool,
            tc.tile_pool(name="bd", bufs=4) as bdpool,
            tc.tile_pool(name="ps_c", bufs=1, space="PSUM") as pcpool,
            tc.tile_pool(name="ps_d", bufs=1, space="PSUM") as pdpool,
            tc.tile_pool(name="ps_main", bufs=2, space="PSUM") as pmpool,
        ):
            # ---- all input DMAs issued up front ----
            # gin0 + consts on Sync (reaches its DMA issues earliest; the
            # Scalar queue has preamble ACT-table loads in front), later
            # guidance supertiles on Scalar, depth pairs on Sync (below).
            gsbs = [gpool.tile([Q, NGMAX * PIX], F16, tag="gsb",
                               name=f"gsb{i}")
                    for i in range(len(SUPER))]
            nc.sync.dma_start(
                out=gsbs[0][:, :SUPER[0] * PIX],
                in_=gin[:, 0:SUPER[0]].rearrange("p g f -> p (g f)"))
            csb = cpool.tile([128, CW], F32, tag="consts")
            nc.sync.dma_start(out=csb, in_=cst)
            g0 = SUPER[0]
            for si, ng in enumerate(SUPER[1:], start=1):
                nc.scalar.dma_start(
                    out=gsbs[si][:, :ng * PIX],
                    in_=gin[:, g0:g0 + ng].rearrange("p g f -> p (g f)"))
                g0 += ng

            # ---- const views ----
            lhsA_v = csb[:, OA:OA + WA].bitcast(F16).rearrange(
                "p (t q) -> p t q", q=Q)
            lhsD_v = csb[:, OD:OD + WD].bitcast(F16).rearrange(
                "p (j q) -> p j q", q=Q)
            lhsD2_v = csb[:, OD2:OD2 + WD].bitcast(F16).rearrange(
                "p (j q) -> p j q", q=Q)
            mask_v = csb[0:Q, OM:OM + WM].bitcast(F16).rearrange(
                "p (a b) -> p a b", b=F)
            convb_v = csb[0:Q, OB:OB + 1]

            # ---- persistent per-core small tensors ----
            craw = spool.tile([Q, NGROUP, F], F32, tag="craw")
            knorm = spool.tile([Q, NGROUP, F], F16, tag="knorm")
            w2 = spool.tile([Q, NGROUP, F], F16, tag="w2")    # 64*W2, fp16
            g24 = spool.tile([Q, NGMAX, P], F16, tag="g24")   # row-sum stage
            r2 = spool.tile([Q, NGROUP], F32, tag="r2")       # r2/64 combined
            r1 = spool.tile([Q, NGROUP], F32, tag="r1")
            sq = spool.tile([Q, NGROUP, F], F32, tag="sq")
            s1 = spool.tile([Q, NGROUP], F32, tag="s1")

            starts = np.cumsum([0] + SUPER)

            def emit_wgen(si):
                """Generator: emits weight-gen for supertile si, yielding
                after each instruction so the caller can interleave it into
                the previous supertile's conv stream (engine queues are
                in-order; emission order IS queue order per engine)."""
                ng = SUPER[si]
                g0 = starts[si]
                gsl = slice(g0, g0 + ng)
                gsb = gsbs[si]

                # gap: two-stage fp16 reduce on Vector (row sums then column
                # sums -- partial sums stay small so fp16 rounding is
                # negligible; 1/576 and x64 folded into the dense weights).
                # Stage 2 writes the fp16 matmul rhs directly; its row 126
                # stays 1.0 from the memset for the K=127 bias rows.
                gap16 = gappool.tile([128, NGMAX], F16, tag="gap16")
                nc.vector.memset(gap16, 1.0)
                with nc.allow_low_precision(
                        reason="two-stage fp16 row/col sums keep partials "
                               "small; tolerance is 2e-2"):
                    for gi in range(ng):
                        nc.vector.tensor_reduce(
                            out=g24[:, gi, :],
                            in_=gsb[:, gi * PIX:(gi + 1) * PIX].rearrange(
                                "p (y x) -> p y x", y=P),
                            axis=AX, op=ADD)
                        yield
                    nc.vector.tensor_reduce(
                        out=gap16[0:Q, :ng], in_=g24[:, :ng, :],
                        axis=AX, op=ADD)
                    yield

                # step A: strided conv -> c, 9 accumulated BD matmuls (fp16)
                psc = pcpool.tile([Q, NGMAX, F], F32, tag="psc")
                gwin = gsb[:, :ng * PIX].rearrange(
                    "p (g oy yr ox xr) -> p g oy ox yr xr",
                    g=ng, oy=KS, yr=8, ox=KS, xr=8)
                for t in range(KS * KS):
                    ky, kx = divmod(t, KS)
                    nc.tensor.matmul(
                        psc[:, :ng, :],
                        lhsT=lhsA_v[0:Q, t, :],
                        rhs=gwin[:, :, :, :, ky, kx],
                        start=(t == 0), stop=(t == KS * KS - 1),
                        skip_group_check=True)
                    yield
                nc.scalar.activation(
                    out=craw[:, gsl, :], in_=psc[:, :ng, :],
                    func=ID, bias=convb_v, scale=1.0)
                yield

                # dense layer, both layouts (D for the norm, D2 for values)
                psD = pdpool.tile([Q, F, NGMAX], F32, tag="psD")
                psD2 = pdpool.tile([Q, F, NGMAX], F32, tag="psD2")
                for j in range(F):
                    nc.tensor.matmul(psD[:, j, :ng], lhsT=lhsD_v[0:Q + 1, j, :],
                                     rhs=gap16[0:Q + 1, :ng],
                                     start=True, stop=True,
                                     skip_group_check=True)
                    yield
                for j in range(F):
                    nc.tensor.matmul(psD2[:, j, :ng], lhsT=lhsD2_v[0:Q + 1, j, :],
                                     rhs=gap16[0:Q + 1, :ng],
                                     start=True, stop=True,
                                     skip_group_check=True)
                    yield

                # r2 = 1/max(||64*W2[:,o]||, 64) == (1/64)/max(||W2||, 1)
                nc.scalar.square(out=sq[:, gsl, :],
                                 in_=psD[:, :, :ng].rearrange("p j g -> p g j"))
                yield
                nc.vector.tensor_reduce(
                    out=s1[:, gsl], in_=sq[:, gsl, :], axis=AX, op=ADD)
                yield
                nc.scalar.sqrt(out=s1[:, gsl], in_=s1[:, gsl])
                yield
                nc.vector.tensor_scalar_max(r2[:, gsl], s1[:, gsl], DSCALE)
                nc.vector.reciprocal(r2[:, gsl], r2[:, gsl])
                yield
                # w2 raw (64x) values, group-major, fp16
                nc.scalar.copy(out=w2[:, gsl, :],
                               in_=psD2[:, :, :ng].rearrange("p o g -> p g o"))
                yield

                # r1 = 1/max(||c||, 1) per (n, ch); knorm = craw * r1
                nc.scalar.square(out=sq[:, gsl, :], in_=craw[:, gsl, :])
                yield
                nc.vector.tensor_reduce(
                    out=s1[:, gsl], in_=sq[:, gsl, :], axis=AX, op=ADD)
                yield
                nc.scalar.sqrt(out=s1[:, gsl], in_=s1[:, gsl])
                yield
                nc.vector.tensor_scalar_max(r1[:, gsl], s1[:, gsl], 1.0)
                nc.vector.reciprocal(r1[:, gsl], r1[:, gsl])
                yield
                nc.vector.tensor_mul(
                    out=knorm[:, gsl, :], in0=craw[:, gsl, :],
                    in1=r1[:, gsl].unsqueeze(2).broadcast_to([Q, ng, F]))
                yield

            def emit_conv_group(g, gl, dsb, osb):
                # w2m = mask (.) W2-row-broadcast  (fp16)
                w2m = bdpool.tile([Q, Q], F16, tag="w2m")
                nc.vector.tensor_mul(
                    out=w2m.rearrange("p (a b) -> p a b", b=F),
                    in0=w2[:, g, :].unsqueeze(1).broadcast_to([Q, NL, F]),
                    in1=mask_v)
                # bd_t = w2m * k_t via broadcast tensor_tensor, taps split
                # across Vector (5) and GpSimd (4) to balance the engines
                bdall = bdpool.tile([Q, KS * KS, Q], F16, tag="bd")
                nc.vector.tensor_tensor(
                    out=bdall[:, 0:6, :],
                    in0=knorm[:, g, 0:6].unsqueeze(2).broadcast_to([Q, 6, Q]),
                    in1=w2m.unsqueeze(1).broadcast_to([Q, 6, Q]),
                    op=MUL)
                nc.gpsimd.tensor_tensor(
                    out=bdall[:, 6:9, :],
                    in0=knorm[:, g, 6:9].unsqueeze(2).broadcast_to([Q, 3, Q]),
                    in1=w2m.unsqueeze(1).broadcast_to([Q, 3, Q]),
                    op=MUL)

                pm = pmpool.tile([Q, 1024], F32, tag="pm")
                drows = dsb[:, gl * PPIX:(gl + 1) * PPIX].rearrange(
                    "p (r c) -> p r c", c=PADW)
                for t in range(KS * KS):
                    ky, kx = divmod(t, KS)
                    for h in range(2):
                        rhs = drows[:, h * 12 + ky:h * 12 + ky + 12,
                                    kx:kx + P]
                        nc.tensor.matmul(
                            pm[:, h * 512:h * 512 + HALF],
                            lhsT=bdall[:, t, :], rhs=rhs,
                            start=(t == 0), stop=(t == KS * KS - 1),
                            skip_group_check=True)

                nc.scalar.activation(
                    out=osb[:, gl * PIX:(gl + 1) * PIX].rearrange(
                        "p (h x) -> p h x", h=2),
                    in_=pm.rearrange("p (h x) -> p h x", h=2)[:, :, 0:HALF],
                    func=CP, bias=0.0, scale=r2[:, g:g + 1])

            def advance(gen, n):
                if gen is None:
                    return None
                for _ in range(n):
                    if next(gen, StopIteration) is StopIteration:
                        return None
                return gen

            # supertile 0's weight-gen runs up front
            for _ in emit_wgen(0):
                pass

            for si, ng in enumerate(SUPER):
                g0 = starts[si]
                # weight-gen of the NEXT supertile, interleaved into this
                # supertile's conv stream (paced after each pair)
                has_next = si + 1 < len(SUPER)
                nxt = emit_wgen(si + 1) if has_next else None
                pairs = [(p0, min(2, g0 + ng - p0))
                         for p0 in range(g0, g0 + ng, 2)]
                quota = -(-(42 + (SUPER[si + 1] if has_next else 0))
                          // len(pairs))

                for p0, npair in pairs:
                    dsb = dpool.tile([Q, 2 * PPIX], F16, tag="dsb")
                    nc.sync.dma_start(
                        out=dsb[:, :npair * PPIX],
                        in_=din[:, p0:p0 + npair].rearrange("p g f -> p (g f)"))
                    osb = opool.tile([Q, 2 * PIX], F16, tag="osb")
                    for gl in range(npair):
                        emit_conv_group(p0 + gl, gl, dsb, osb)
                    nc.gpsimd.dma_start(
                        out=outd[:, p0:p0 + npair].rearrange("p g f -> p (g f)"),
                        in_=osb[:, :npair * PIX])
                    nxt = advance(nxt, quota)

                if nxt is not None:  # drain any leftovers
                    for _ in nxt:
                        pass

    nc.compile()
    return nc


def _host_prep(guidance, depth, conv_w, conv_b, dense_w, dense_b):
    B, H, W, _ = guidance.shape
    nh, nw = H // P, W // P
    NB = B * nh * nw

    def to_samples(x):
        # (B,H,W,F) -> (NB, P, P, F), sample order = flat (b, i, j)
        return (x.reshape(B, nh, P, nw, P, F)
                 .transpose(0, 1, 3, 2, 4, 5)
                 .reshape(NB, P, P, F))

    gs = to_samples(np.ascontiguousarray(guidance))
    ds = to_samples(np.ascontiguousarray(depth))

    in_maps = []
    for c in range(NCORES):
        gsl = gs[c * 512:(c + 1) * 512]
        dsl = ds[c * 512:(c + 1) * 512]
        gpad = np.zeros((SPC, P, P, F), np.float32)
        gpad[:512] = gsl
        dpad = np.zeros((SPC, PADW, PADW, F), np.float32)
        dpad[:512, 1:P + 1, 1:P + 1] = dsl
        # (SPC, y, x, ch) -> [126, NGROUP, pix]  with q = n_local*9 + ch
        gq = (gpad.reshape(NGROUP, NL, P, P, F)
                  .transpose(1, 4, 0, 2, 3)
                  .reshape(Q, NGROUP, PIX))
        dq = (dpad.reshape(NGROUP, NL, PADW, PADW, F)
                  .transpose(1, 4, 0, 2, 3)
                  .reshape(Q, NGROUP, PPIX))
        in_maps.append({"gin": np.ascontiguousarray(gq.astype(np.float16)),
                        "din": np.ascontiguousarray(dq.astype(np.float16))})

    eye = np.eye(NL, dtype=np.float32)
    lhsA = np.zeros((128, KS * KS, Q), np.float16)
    for t in range(KS * KS):
        ky, kx = divmod(t, KS)
        lhsA[0:Q, t, :] = np.kron(eye, conv_w[ky, kx]).astype(np.float16)
    lhsD = np.zeros((128, KS * KS, Q), np.float16)
    lhsD2 = np.zeros((128, KS * KS, Q), np.float16)
    dws = dense_w.astype(np.float32) * (DSCALE / PIX)
    dbs = dense_b.astype(np.float32) * DSCALE
    for j in range(F):
        lhsD[0:Q, j, :] = np.kron(eye, dws[:, j * F:(j + 1) * F])
        lhsD[Q, j, :] = np.tile(dbs[j * F:(j + 1) * F], NL)
        lhsD2[0:Q, j, :] = np.kron(eye, dws[:, j::F])
        lhsD2[Q, j, :] = np.tile(dbs[j::F], NL)
    mask = np.zeros((128, Q), np.float16)
    mask[0:Q] = np.kron(eye, np.ones((F, F), np.float32))
    convb = np.zeros((128, 1), np.float32)
    convb[0:Q, 0] = np.tile(conv_b.astype(np.float32), NL)

    def f16words(a):
        return np.ascontiguousarray(a.reshape(128, -1)).view(np.float32)

    consts = np.concatenate(
        [f16words(lhsA), f16words(lhsD), f16words(lhsD2), f16words(mask),
         convb], axis=1)
    assert consts.shape == (128, CW), consts.shape

    consts = np.ascontiguousarray(consts)
    for m in in_maps:
        m["consts"] = consts
    return in_maps


_CACHED_NC = None


def run(inputs, trace=False, **kw):
    """Build (cached), run on 8 cores, return (full_output, BassKernelResults)."""
    global _CACHED_NC
    inputs = {k: np.asarray(v, np.float32) for k, v in inputs.items()}
    in_maps = _host_prep(**inputs)
    if _CACHED_NC is None:
        _CACHED_NC = build_program()
    res = run_bass_kernel_spmd(_CACHED_NC, in_maps, list(range(NCORES)),
                               trace=trace, **kw)
    outs = []
    for c in range(NCORES):
        o = res.results[c]["out"].astype(np.float32)
        o = o.reshape(NL, F, NGROUP, P, P)
        o = o.transpose(2, 0, 3, 4, 1).reshape(SPC, P, P, F)[:512]
        outs.append(o)
    full = np.concatenate(outs, 0)  # (4096, 24, 24, 9) in (b, i, j) order
    B, H, W = 16, 384, 384
    return full.reshape(B, H, W, F), res


def kernel(**inputs):
    out, _ = run(inputs, trace=False)
    return out
